# revision 49
# baseline (speedup 1.0000x reference)
"""Single-head attention (b=4, s=4096, d=1024, h=64) on 8 TRN2 NeuronCores.

Sharding: core c handles batch c//2, query half c%2 (2048 queries), with the
full 4096-key context of that batch. No collectives needed. The host
transposes x[b] to x^T [1024, 4096] (bf16) with the core's query columns
rotated to the front, so one SPMD graph serves all 8 cores (softmax is
permutation-invariant over keys).

Pipeline (all matmuls bf16, f32 PSUM):
  Q^T = Wq^T x^T  [64, 2048] upfront (PSUM partitions 64..127 via
        tile_position so Q lands on the same partitions K uses).
  qh=0 pass, per key tile t (128 keys):
    KV^T[t] = [Wv|Wk]^T x^T[:, t]  -> PSUM [128, 128]  (V rows 0..63,
              K rows 64..127; packed so the PE runs a full 128-wide output)
    kt[t] <- rows 64..127 (bf16), vt <- rows 0..63, PE-transpose vt to
              V[t] [128 keys, 64] and append a ones column -> V_aug
    S^T[t] = K[t].T Q^T[:, 0:1024] -> PSUM [128, 1024]
    P^T[t] = exp(0.125 S^T[t])     -> bf16 (ScalarE LUT, no max needed)
    O_aug^T += V_aug[t].T P^T[t]   -> PSUM [65, 1024] accumulated over t
  qh=1 pass: same minus the projections (Act-engine bound).
  Epilogue per qh: PE-transpose O_aug^T back to [queries, 65], divide by
  the ones-row (softmax denominator), DMA out f32. qh=0's epilogue
  overlaps the qh=1 main loop.
"""

import sys

for _p in ("/opt/trn_rl_repo",):
    if _p not in sys.path:
        sys.path.insert(0, _p)

from contextlib import ExitStack

import numpy as np
import ml_dtypes

import concourse.bass as bass
import concourse.tile as tile
from concourse import mybir
from concourse.bass_utils import run_bass_kernel_spmd
from concourse.masks import make_identity

BF16 = mybir.dt.bfloat16
F32 = mybir.dt.float32

B, S, D, H = 4, 4096, 1024, 64
NCORES = 8
SQ = S // 2          # queries per core
DC = D // 128        # d contraction chunks
ST = S // 128        # key tiles
QW = 1024            # query width per pass (PSUM budget)
NQH = SQ // QW

_CACHE = {}


def _build_nc(reps=1):
    nc = bass.Bass("TRN2", target_bir_lowering=False, debug=False,
                   num_devices=NCORES)
    xT_d = nc.dram_tensor("xT", [D, S], BF16, kind="ExternalInput")
    wvk_d = nc.dram_tensor("wvk", [D, 128], BF16, kind="ExternalInput")
    wq_d = nc.dram_tensor("wq", [D, H], BF16, kind="ExternalInput")
    out_d = nc.dram_tensor("out", [SQ, H], F32, kind="ExternalOutput")

    with tile.TileContext(nc) as tc, ExitStack() as ctx:
        _emit(ctx, tc, nc, xT_d.ap(), wvk_d.ap(), wq_d.ap(), out_d.ap(),
              reps=reps)
    _split_matmul_waits(nc)
    return nc


_SPLIT_OPS = ("Matmult", "Activation", "TensorCopy", "TensorScalarPtr",
              "TensorTensor", "TensorReduce", "Reciprocal", "Memset",
              "Ldweights", "TensorScalarAffineSelect", "Iota",
              "CopyPredicated", "StreamTranspose", "DMACopy", "Drain",
              "NoOp")


def _split_matmul_waits(nc):
    """The 64B compute-instruction encodings hold a single sync wait; Tile
    occasionally attaches two. Hoist the extras onto standalone
    EventSemaphore instructions placed just before the instruction in the
    same engine stream (waits are >=-monotone, so waiting earlier is always
    safe)."""
    n = 0
    for f in nc.m.functions:
        for b in f.blocks:
            new_insts = []
            for i in b.instructions:
                si = getattr(i, "sync_info", None)
                if (i.opcode in _SPLIT_OPS and si is not None and si.on_wait
                        and len(si.on_wait) > 1):
                    for w in list(si.on_wait[:-1]):
                        n += 1
                        ev = mybir.InstEventSemaphore(
                            name=f"I-mmwait-{n}",
                            opcode="EventSemaphore",
                            engine=i.engine,
                            ins=[], outs=[],
                            sync_info=mybir.SyncInfo(on_wait=[w],
                                                     on_update=[]),
                        )
                        new_insts.append(ev)
                    i.sync_info = mybir.SyncInfo(on_wait=[si.on_wait[-1]],
                                                 on_update=si.on_update)
                new_insts.append(i)
            b.instructions = new_insts


def _emit(ctx, tc, nc, xT, wvk, wq, out, reps=1):
    P = 128
    NSL = 8              # x arrives in 8 seq slices of 512 positions
    SL = S // NSL
    Exp = mybir.ActivationFunctionType.Exp

    xt_pool = ctx.enter_context(tc.tile_pool(name="xt", bufs=1))
    w_pool = ctx.enter_context(tc.tile_pool(name="w", bufs=1))
    vt_pool = ctx.enter_context(tc.tile_pool(name="vt", bufs=2))
    pt_pool = ctx.enter_context(tc.tile_pool(name="pt", bufs=4))
    epi_pool = ctx.enter_context(tc.tile_pool(name="epi", bufs=2))

    # --- load inputs, sequence-sliced so compute starts early ----------
    # DMA order: wq, slice0, wvk, slices 1..7 (the DMA engines are a
    # serialized ~360 GB/s resource; Q chunk 0 + the first key tiles must
    # not wait for the whole 8 MiB of x).
    wq_sb = w_pool.tile([P, DC, H], BF16, tag="wq")
    wvk_sb = w_pool.tile([P, DC, P], BF16, tag="wvk")
    xt = [xt_pool.tile([P, DC, SL], BF16, tag=f"xt{sl}", name=f"xts{sl}")
          for sl in range(NSL)]

    def load_x():
        # weights first, then cols 0..1023 (q chunks + first key groups) in
        # 256-col DMAs so the Q projection and first projections start
        # early, then the remaining 512-col slices
        nc.sync.dma_start(wq_sb, wq.rearrange("(o p) h -> p o h", p=P))
        nc.sync.dma_start(wvk_sb, wvk.rearrange("(o p) h -> p o h", p=P))
        pieces = [(q4 // 2, q4 * 256, (q4 + 1) * 256) for q4 in range(4)]
        pieces += [(sl, sl * SL, (sl + 1) * SL) for sl in range(2, NSL)]
        for sl, c0, c1 in pieces:
            nc.sync.dma_start(
                xt[sl][:, :, c0 - sl * SL:c1 - sl * SL],
                xT[:, c0:c1].rearrange("(o p) s -> p o s", p=P))

    def xs(dc, c0, c1):  # x^T[dc*128:(dc+1)*128, c0:c1] from the slice tiles
        sl = c0 // SL
        assert c1 <= (sl + 1) * SL
        return xt[sl][:, dc, c0 - sl * SL:c1 - sl * SL]

    ident = w_pool.tile([H + 1, H + 1], F32, tag="ident")
    make_identity(nc, ident)

    # persistent SBUF state: K^T and Q^T on partitions 64..127, V_aug with
    # keys on partitions
    kt_sb = w_pool.tile([P, S], BF16, tag="kt")      # rows 64..127 used
    qt_sb = w_pool.tile([P, SQ], BF16, tag="qt")     # rows 64..127 used
    v_sb = w_pool.tile([P, ST, H + 1], BF16, tag="v")
    nc.vector.memset(v_sb[:, :, H:H + 1], 1.0)
    # zero the unused partition halves ONCE (outside the rep loop): the
    # score matmuls can then contract over all 128 rows -- zero weight
    # rows contribute exactly 0, but the full-array matmul runs at 2.4GHz
    # instead of the half-array 1.2GHz clock (HW-measured 475 -> ~320 ns
    # per 512-col matmul)
    nc.vector.memset(kt_sb[0:H, :], 0.0)
    nc.vector.memset(qt_sb[0:H, :], 0.0)

    ps_s = ctx.enter_context(tc.tile_pool(name="pss", bufs=2, space="PSUM"))
    ps_o = ctx.enter_context(tc.tile_pool(name="pso", bufs=1, space="PSUM"))
    ps_a = ctx.enter_context(tc.tile_pool(name="psa", bufs=2, space="PSUM"))

    def q_proj(c0, c1):
        # Q cols [c0, c1) -> qt_sb[64:128, c0:c1] (PSUM partitions 64..127
        # via tile_position so Q lands where K lives)
        ps = ps_a.tile([P, 512], F32, tag="kvps")
        w = c1 - c0
        for dc in range(DC):
            nc.tensor.matmul(ps[H:P, 0:w], lhsT=wq_sb[:, dc, :],
                             rhs=xs(dc, c0, c1),
                             start=(dc == 0), stop=(dc == DC - 1),
                             tile_position=(0, 64))
        nc.vector.tensor_copy(out=qt_sb[H:P, c0:c1], in_=ps[H:P, 0:w])

    def kv_proj_mm(g, dc0, dc1, kv=None):
        # packed [Wv|Wk] projection for the 4 key tiles at column g*512;
        # emits dc chunks [dc0, dc1) so the 8 accumulation matmuls can be
        # interlaced with scores work. Group width keeps each Ldweights
        # serving 512 moving columns.
        if kv is None:
            kv = ps_a.tile([P, 512], F32, tag="kvps")
        c0 = g * 512
        for dc in range(dc0, dc1):
            nc.tensor.matmul(kv[:, 0:512], lhsT=wvk_sb[:, dc, :],
                             rhs=xs(dc, c0, c0 + 512),
                             start=(dc == 0), stop=(dc == DC - 1),
                             skip_group_check=True)
        return kv

    def kv_copies(g, kv):
        c0 = g * 512
        nc.vector.tensor_copy(out=kt_sb[H:P, c0:c0 + 512], in_=kv[H:P, :])
        vt = vt_pool.tile([H, 512], F32, tag="vt")
        nc.vector.tensor_copy(out=vt, in_=kv[0:H, :])
        return vt

    def kv_proj(g):
        return kv_copies(g, kv_proj_mm(g, 0, DC))

    def v_trans(g, vt, w=512):
        # transpose the group's V^T [64, w] into V_aug [keys, 64] tiles
        tr = ps_a.tile([P, 512], F32, tag="kvps")
        for j in range(w // P):
            nc.tensor.transpose(tr[:, j * H:(j + 1) * H],
                                vt[:, j * P:(j + 1) * P], ident[0:H, 0:H])
        t0 = g * 4
        nc.vector.tensor_copy(
            out=v_sb[:, t0:t0 + w // P, 0:H],
            in_=tr[:, 0:(w // P) * H].rearrange("p (t h) -> p t h", h=H))

    def s_exp(qh, t):
        s_ps = ps_s.tile([P, QW], F32, tag="sps")
        for qc in range(QW // 512):
            # full 128-row contraction: rows 0..63 of kt are zero, so the
            # result is unchanged but the PE runs at full (8/8) clock
            nc.tensor.matmul(
                s_ps[:, qc * 512:(qc + 1) * 512],
                lhsT=kt_sb[:, t * P:(t + 1) * P],
                rhs=qt_sb[:, qh * QW + qc * 512: qh * QW + (qc + 1) * 512],
                start=True, stop=True)
        # 1/sqrt(h) is folded into Wq host-side (exact power of 2), so the
        # activation runs scale-free: ~1.22us vs ~1.53us per [128,1024] exp
        pt = pt_pool.tile([P, QW], BF16, tag="pt")
        nc.scalar.activation(pt, s_ps, Exp)
        return pt

    def av(o_ps, t, pt):
        for qc in range(QW // 512):
            nc.tensor.matmul(
                o_ps[:, qc * 512:(qc + 1) * 512],
                lhsT=v_sb[:, t, :],
                rhs=pt[:, qc * 512:(qc + 1) * 512],
                start=(t == 0), stop=(t == ST - 1))

    Copy = mybir.ActivationFunctionType.Copy

    def epi_start(qh, o_ps):
        # PE-transpose O_aug^T back to [queries, 65] per 128-query block;
        # DVE takes the reciprocal of the denominator row and the (idle)
        # Act engine applies it, staging results so the store is one DMA
        # per 512-query half.
        ot_sb = epi_pool.tile([H + 1, QW], F32, tag="ot")
        stage = epi_pool.tile([P, QW // P, H], F32, tag="stage")
        for half in range(2):
            nc.vector.tensor_copy(
                out=ot_sb[:, half * 512:(half + 1) * 512],
                in_=o_ps[:, half * 512:(half + 1) * 512])
        return ot_sb, stage

    def epi_block(qh, ot_sb, stage, qt, on_act=True):
        # the normalize runs on Act only when Act is idle (the final tail);
        # the qh=0 epilogue rides inside the Act-bound qh=1 loop, so there
        # it uses DVE slack instead
        tr_ps = ps_a.tile([P, 512], F32, tag="kvps")
        nc.tensor.transpose(tr_ps[:, 0:H + 1],
                            ot_sb[:, qt * P:(qt + 1) * P], ident)
        rec = epi_pool.tile([P, 1], F32, tag="rec")
        nc.vector.reciprocal(rec, tr_ps[:, H:H + 1])
        if on_act:
            nc.scalar.activation(stage[:, qt, :], tr_ps[:, 0:H], Copy,
                                 scale=rec)
        else:
            nc.vector.tensor_scalar_mul(stage[:, qt, :], tr_ps[:, 0:H], rec)
        if qt in (3, QW // P - 1):
            h0 = 0 if qt == 3 else 512
            nc.sync.dma_start(
                out[qh * QW + h0:qh * QW + h0 + 512, :]
                .rearrange("(o p) h -> p o h", p=P),
                stage[:, h0 // P:h0 // P + 4, :])

    def epilogue(qh, o_ps):
        # DVE normalize even in the tail: Act's Copy with an AP scale pays
        # the ~306ns scale penalty per block, DVE's tensor_scalar is ~3x
        # cheaper and the tail DVE is idle
        ot_sb, stage = epi_start(qh, o_ps)
        for qt in range(QW // P):
            epi_block(qh, ot_sb, stage, qt, on_act=False)

    # --- attention, software-pipelined ---------------------------------
    # qh=0 carries the K/V projections: step i projects tile i, scores
    # tile i-1, and applies attention-V for tile i-2, so every cross-engine
    # dependency has a full step of slack. Q chunks for qh=0 run first in
    # 256-col pieces behind their x DMAs; qh=1's chunks slot into early
    # steps once their x slices land.
    warm_sb = w_pool.tile([P, 1], F32, tag="warm")
    warm_out = pt_pool.tile([P, 1], BF16, tag="warmo")
    nc.gpsimd.memset(warm_sb, 0.0)

    def warmup_pe():
        # ~1.2us of back-to-back PE work so the tensor engine leaves its
        # low/mid p-state before the first projection lands, and a dummy
        # 1-col exp so the Act engine's 1.3us LUT table load happens in
        # the DMA ramp instead of inside the first critical-path exp
        nc.scalar.activation(warm_out, warm_sb, Exp)
        for _ in range(3):
            wps = ps_a.tile([P, 512], F32, tag="kvps")
            for r in range(7):
                nc.tensor.transpose(wps[0:H + 1, r * (H + 1):(r + 1) * (H + 1)],
                                    ident, ident)

    def body(with_dma=True):
        NG = ST // 4     # key-tile groups of 4 (512 cols) per kv projection
        if with_dma:
            load_x()
        warmup_pe()
        # ramp: Q quarters interleaved with the first two kv groups, all
        # gated only on the first four 256-col x DMAs
        # ramp: Q quarters + the first kv group, gated only on the 256-col
        # x DMAs; group g's projection then rides in step g+1 and its
        # V-transpose in step g+2, one step ahead of use
        vts = {}
        q_proj(0, 256)
        q_proj(256, 512)
        vts[0] = kv_proj(0)
        q_proj(512, 768)
        q_proj(768, 1024)
        o_ps0 = ps_o.tile([H + 1, QW], F32, tag="ops")
        prev0 = None
        for g in range(2, NG + 2):
            kv = kv_proj_mm(g - 1, 0, 2) if g - 1 < NG else None
            for j in range(4):
                t = (g - 2) * 4 + j
                pt = s_exp(0, t)
                if kv is not None and j < 3:
                    kv_proj_mm(g - 1, 2 * (j + 1), 2 * (j + 2), kv)
                if j == 0 and g - 2 in vts:
                    v_trans(g - 2, vts.pop(g - 2))
                if prev0 is not None:
                    av(o_ps0, prev0[0], prev0[1])
                prev0 = (t, pt)
            if kv is not None:
                vts[g - 1] = kv_copies(g - 1, kv)
            if g == 2:
                q_proj(1024, 1536)
            if g == 3:
                q_proj(1536, 2048)
        av(o_ps0, prev0[0], prev0[1])

        # qh=1 main loop; qh=0's epilogue blocks ride in its PE/DVE slack
        o_ps1 = ps_o.tile([H + 1, QW], F32, tag="ops")
        epi0 = None
        prev = None
        for t in range(ST):
            pt = s_exp(1, t)
            if t == 0:
                epi0 = epi_start(0, o_ps0)
            if prev is not None:
                av(o_ps1, prev[0], prev[1])
            if 1 <= t <= QW // P:
                epi_block(0, epi0[0], epi0[1], t - 1, on_act=False)
            prev = (t, pt)
        av(o_ps1, prev[0], prev[1])
        epilogue(1, o_ps1)

    import os
    dma_once = bool(int(os.environ.get("KBENCH_DMA_ONCE", "0")))
    if reps == 1:
        body()
    else:
        if dma_once:
            load_x()
        with tc.For_i(0, reps):
            body(with_dma=not dma_once)


def _get_nc():
    if "nc" not in _CACHE:
        _CACHE["nc"] = _build_nc()
    return _CACHE["nc"]


def kernel(x, Wk, Wq, Wv, _trace=False):
    x = np.asarray(x, dtype=np.float32)
    bf = ml_dtypes.bfloat16
    wvk = np.concatenate([np.asarray(Wv, np.float32),
                          np.asarray(Wk, np.float32)], axis=1).astype(bf)
    # fold the 1/sqrt(HEAD_SIZE) score scale into Wq (exact in bf16)
    wq = (np.asarray(Wq, dtype=np.float32) * 0.125).astype(bf)

    in_maps = []
    for c in range(NCORES):
        b, qh = divmod(c, 2)
        xb = x[b]
        if qh:
            xb = np.concatenate([xb[SQ:], xb[:SQ]], axis=0)
        xT = np.ascontiguousarray(xb.T).astype(bf)
        in_maps.append({"xT": xT, "wvk": wvk, "wq": wq})

    nc = _get_nc()
    res = run_bass_kernel_spmd(nc, in_maps, core_ids=list(range(NCORES)),
                               trace=_trace)
    _CACHE["last_result"] = res

    out = np.empty((B, S, H), np.float32)
    for c in range(NCORES):
        b, qh = divmod(c, 2)
        out[b, qh * SQ:(qh + 1) * SQ, :] = res.results[c]["out"]
    return out



# revision 50
# speedup vs baseline: 1.0059x; 1.0059x over previous
"""Single-head attention (b=4, s=4096, d=1024, h=64) on 8 TRN2 NeuronCores.

Sharding: core c handles batch c//2, query half c%2 (2048 queries), with the
full 4096-key context of that batch. No collectives needed. The host
transposes x[b] to x^T [1024, 4096] (bf16) with the core's query columns
rotated to the front, so one SPMD graph serves all 8 cores (softmax is
permutation-invariant over keys).

Pipeline (all matmuls bf16, f32 PSUM):
  Q^T = Wq^T x^T  [64, 2048] upfront (PSUM partitions 64..127 via
        tile_position so Q lands on the same partitions K uses).
  qh=0 pass, per key tile t (128 keys):
    KV^T[t] = [Wv|Wk]^T x^T[:, t]  -> PSUM [128, 128]  (V rows 0..63,
              K rows 64..127; packed so the PE runs a full 128-wide output)
    kt[t] <- rows 64..127 (bf16), vt <- rows 0..63, PE-transpose vt to
              V[t] [128 keys, 64] and append a ones column -> V_aug
    S^T[t] = K[t].T Q^T[:, 0:1024] -> PSUM [128, 1024]
    P^T[t] = exp(0.125 S^T[t])     -> bf16 (ScalarE LUT, no max needed)
    O_aug^T += V_aug[t].T P^T[t]   -> PSUM [65, 1024] accumulated over t
  qh=1 pass: same minus the projections (Act-engine bound).
  Epilogue per qh: PE-transpose O_aug^T back to [queries, 65], divide by
  the ones-row (softmax denominator), DMA out f32. qh=0's epilogue
  overlaps the qh=1 main loop.
"""

import sys

for _p in ("/opt/trn_rl_repo",):
    if _p not in sys.path:
        sys.path.insert(0, _p)

from contextlib import ExitStack

import numpy as np
import ml_dtypes

import concourse.bass as bass
import concourse.tile as tile
from concourse import mybir
from concourse.bass_utils import run_bass_kernel_spmd
from concourse.masks import make_identity

BF16 = mybir.dt.bfloat16
F32 = mybir.dt.float32

B, S, D, H = 4, 4096, 1024, 64
NCORES = 8
SQ = S // 2          # queries per core
DC = D // 128        # d contraction chunks
ST = S // 128        # key tiles
QW = 1024            # query width per pass (PSUM budget)
NQH = SQ // QW

_CACHE = {}


def _build_nc(reps=1):
    nc = bass.Bass("TRN2", target_bir_lowering=False, debug=False,
                   num_devices=NCORES)
    xT_d = nc.dram_tensor("xT", [D, S], BF16, kind="ExternalInput")
    wvk_d = nc.dram_tensor("wvk", [D, 128], BF16, kind="ExternalInput")
    wq_d = nc.dram_tensor("wq", [D, H], BF16, kind="ExternalInput")
    out_d = nc.dram_tensor("out", [SQ, H], F32, kind="ExternalOutput")

    with tile.TileContext(nc) as tc, ExitStack() as ctx:
        _emit(ctx, tc, nc, xT_d.ap(), wvk_d.ap(), wq_d.ap(), out_d.ap(),
              reps=reps)
    _split_matmul_waits(nc)
    return nc


_SPLIT_OPS = ("Matmult", "Activation", "TensorCopy", "TensorScalarPtr",
              "TensorTensor", "TensorReduce", "Reciprocal", "Memset",
              "Ldweights", "TensorScalarAffineSelect", "Iota",
              "CopyPredicated", "StreamTranspose", "DMACopy", "Drain",
              "NoOp")


def _split_matmul_waits(nc):
    """The 64B compute-instruction encodings hold a single sync wait; Tile
    occasionally attaches two. Hoist the extras onto standalone
    EventSemaphore instructions placed just before the instruction in the
    same engine stream (waits are >=-monotone, so waiting earlier is always
    safe)."""
    n = 0
    for f in nc.m.functions:
        for b in f.blocks:
            new_insts = []
            for i in b.instructions:
                si = getattr(i, "sync_info", None)
                if (i.opcode in _SPLIT_OPS and si is not None and si.on_wait
                        and len(si.on_wait) > 1):
                    for w in list(si.on_wait[:-1]):
                        n += 1
                        ev = mybir.InstEventSemaphore(
                            name=f"I-mmwait-{n}",
                            opcode="EventSemaphore",
                            engine=i.engine,
                            ins=[], outs=[],
                            sync_info=mybir.SyncInfo(on_wait=[w],
                                                     on_update=[]),
                        )
                        new_insts.append(ev)
                    i.sync_info = mybir.SyncInfo(on_wait=[si.on_wait[-1]],
                                                 on_update=si.on_update)
                new_insts.append(i)
            b.instructions = new_insts


def _emit(ctx, tc, nc, xT, wvk, wq, out, reps=1):
    P = 128
    NSL = 8              # x arrives in 8 seq slices of 512 positions
    SL = S // NSL
    Exp = mybir.ActivationFunctionType.Exp

    xt_pool = ctx.enter_context(tc.tile_pool(name="xt", bufs=1))
    w_pool = ctx.enter_context(tc.tile_pool(name="w", bufs=1))
    vt_pool = ctx.enter_context(tc.tile_pool(name="vt", bufs=2))
    pt_pool = ctx.enter_context(tc.tile_pool(name="pt", bufs=4))
    epi_pool = ctx.enter_context(tc.tile_pool(name="epi", bufs=2))

    # --- load inputs, sequence-sliced so compute starts early ----------
    # DMA order: wq, slice0, wvk, slices 1..7 (the DMA engines are a
    # serialized ~360 GB/s resource; Q chunk 0 + the first key tiles must
    # not wait for the whole 8 MiB of x).
    wq_sb = w_pool.tile([P, DC, H], BF16, tag="wq")
    wvk_sb = w_pool.tile([P, DC, P], BF16, tag="wvk")
    xt = [xt_pool.tile([P, DC, SL], BF16, tag=f"xt{sl}", name=f"xts{sl}")
          for sl in range(NSL)]

    def load_x():
        # weights first, then cols 0..1023 (q chunks + first key groups) in
        # 256-col DMAs so the Q projection and first projections start
        # early, then the remaining 512-col slices
        nc.sync.dma_start(wq_sb, wq.rearrange("(o p) h -> p o h", p=P))
        nc.sync.dma_start(wvk_sb, wvk.rearrange("(o p) h -> p o h", p=P))
        pieces = [(q4 // 2, q4 * 256, (q4 + 1) * 256) for q4 in range(4)]
        pieces += [(sl, sl * SL, (sl + 1) * SL) for sl in range(2, NSL)]
        for sl, c0, c1 in pieces:
            nc.sync.dma_start(
                xt[sl][:, :, c0 - sl * SL:c1 - sl * SL],
                xT[:, c0:c1].rearrange("(o p) s -> p o s", p=P))

    def xs(dc, c0, c1):  # x^T[dc*128:(dc+1)*128, c0:c1] from the slice tiles
        sl = c0 // SL
        assert c1 <= (sl + 1) * SL
        return xt[sl][:, dc, c0 - sl * SL:c1 - sl * SL]

    ident = w_pool.tile([H + 1, H + 1], F32, tag="ident")
    make_identity(nc, ident)

    # persistent SBUF state: K^T and Q^T on partitions 64..127, V_aug with
    # keys on partitions
    kt_sb = w_pool.tile([P, S], BF16, tag="kt")      # rows 64..127 used
    qt_sb = w_pool.tile([P, SQ], BF16, tag="qt")     # rows 64..127 used
    v_sb = w_pool.tile([P, ST, H + 1], BF16, tag="v")
    nc.vector.memset(v_sb[:, :, H:H + 1], 1.0)

    ps_s = ctx.enter_context(tc.tile_pool(name="pss", bufs=2, space="PSUM"))
    ps_o = ctx.enter_context(tc.tile_pool(name="pso", bufs=1, space="PSUM"))
    ps_a = ctx.enter_context(tc.tile_pool(name="psa", bufs=2, space="PSUM"))

    def q_proj(c0, c1):
        # Q cols [c0, c1) -> qt_sb[64:128, c0:c1] (PSUM partitions 64..127
        # via tile_position so Q lands where K lives)
        ps = ps_a.tile([P, 512], F32, tag="kvps")
        w = c1 - c0
        for dc in range(DC):
            nc.tensor.matmul(ps[H:P, 0:w], lhsT=wq_sb[:, dc, :],
                             rhs=xs(dc, c0, c1),
                             start=(dc == 0), stop=(dc == DC - 1),
                             tile_position=(0, 64))
        nc.vector.tensor_copy(out=qt_sb[H:P, c0:c1], in_=ps[H:P, 0:w])

    def kv_proj_mm(g, dc0, dc1, kv=None):
        # packed [Wv|Wk] projection for the 4 key tiles at column g*512;
        # emits dc chunks [dc0, dc1) so the 8 accumulation matmuls can be
        # interlaced with scores work. Group width keeps each Ldweights
        # serving 512 moving columns.
        if kv is None:
            kv = ps_a.tile([P, 512], F32, tag="kvps")
        c0 = g * 512
        for dc in range(dc0, dc1):
            nc.tensor.matmul(kv[:, 0:512], lhsT=wvk_sb[:, dc, :],
                             rhs=xs(dc, c0, c0 + 512),
                             start=(dc == 0), stop=(dc == DC - 1),
                             skip_group_check=True)
        return kv

    def kv_copies(g, kv):
        c0 = g * 512
        nc.vector.tensor_copy(out=kt_sb[H:P, c0:c0 + 512], in_=kv[H:P, :])
        vt = vt_pool.tile([H, 512], F32, tag="vt")
        nc.vector.tensor_copy(out=vt, in_=kv[0:H, :])
        return vt

    def kv_proj(g):
        return kv_copies(g, kv_proj_mm(g, 0, DC))

    def v_trans(g, vt, w=512):
        # transpose the group's V^T [64, w] into V_aug [keys, 64] tiles
        tr = ps_a.tile([P, 512], F32, tag="kvps")
        for j in range(w // P):
            nc.tensor.transpose(tr[:, j * H:(j + 1) * H],
                                vt[:, j * P:(j + 1) * P], ident[0:H, 0:H])
        t0 = g * 4
        nc.vector.tensor_copy(
            out=v_sb[:, t0:t0 + w // P, 0:H],
            in_=tr[:, 0:(w // P) * H].rearrange("p (t h) -> p t h", h=H))

    def s_exp(qh, t):
        s_ps = ps_s.tile([P, QW], F32, tag="sps")
        for qc in range(QW // 512):
            nc.tensor.matmul(
                s_ps[:, qc * 512:(qc + 1) * 512],
                lhsT=kt_sb[H:P, t * P:(t + 1) * P],
                rhs=qt_sb[H:P, qh * QW + qc * 512: qh * QW + (qc + 1) * 512],
                start=True, stop=True)
        # 1/sqrt(h) is folded into Wq host-side (exact power of 2), so the
        # activation runs scale-free: ~1.22us vs ~1.53us per [128,1024] exp
        pt = pt_pool.tile([P, QW], BF16, tag="pt")
        nc.scalar.activation(pt, s_ps, Exp)
        return pt

    def av(o_ps, t, pt):
        for qc in range(QW // 512):
            nc.tensor.matmul(
                o_ps[:, qc * 512:(qc + 1) * 512],
                lhsT=v_sb[:, t, :],
                rhs=pt[:, qc * 512:(qc + 1) * 512],
                start=(t == 0), stop=(t == ST - 1))

    Copy = mybir.ActivationFunctionType.Copy

    def epi_start(qh, o_ps):
        # PE-transpose O_aug^T back to [queries, 65] per 128-query block;
        # DVE takes the reciprocal of the denominator row and the (idle)
        # Act engine applies it, staging results so the store is one DMA
        # per 512-query half.
        ot_sb = epi_pool.tile([H + 1, QW], F32, tag="ot")
        stage = epi_pool.tile([P, QW // P, H], F32, tag="stage")
        for half in range(2):
            nc.vector.tensor_copy(
                out=ot_sb[:, half * 512:(half + 1) * 512],
                in_=o_ps[:, half * 512:(half + 1) * 512])
        return ot_sb, stage

    def epi_block(qh, ot_sb, stage, qt, on_act=True):
        # the normalize runs on Act only when Act is idle (the final tail);
        # the qh=0 epilogue rides inside the Act-bound qh=1 loop, so there
        # it uses DVE slack instead
        tr_ps = ps_a.tile([P, 512], F32, tag="kvps")
        nc.tensor.transpose(tr_ps[:, 0:H + 1],
                            ot_sb[:, qt * P:(qt + 1) * P], ident)
        rec = epi_pool.tile([P, 1], F32, tag="rec")
        nc.vector.reciprocal(rec, tr_ps[:, H:H + 1])
        if on_act:
            nc.scalar.activation(stage[:, qt, :], tr_ps[:, 0:H], Copy,
                                 scale=rec)
        else:
            nc.vector.tensor_scalar_mul(stage[:, qt, :], tr_ps[:, 0:H], rec)
        if qt in (3, QW // P - 1):
            h0 = 0 if qt == 3 else 512
            nc.sync.dma_start(
                out[qh * QW + h0:qh * QW + h0 + 512, :]
                .rearrange("(o p) h -> p o h", p=P),
                stage[:, h0 // P:h0 // P + 4, :])

    def epilogue(qh, o_ps):
        # DVE normalize even in the tail: Act's Copy with an AP scale pays
        # the ~306ns scale penalty per block, DVE's tensor_scalar is ~3x
        # cheaper and the tail DVE is idle
        ot_sb, stage = epi_start(qh, o_ps)
        for qt in range(QW // P):
            epi_block(qh, ot_sb, stage, qt, on_act=False)

    # --- attention, software-pipelined ---------------------------------
    # qh=0 carries the K/V projections: step i projects tile i, scores
    # tile i-1, and applies attention-V for tile i-2, so every cross-engine
    # dependency has a full step of slack. Q chunks for qh=0 run first in
    # 256-col pieces behind their x DMAs; qh=1's chunks slot into early
    # steps once their x slices land.
    warm_sb = w_pool.tile([P, 1], F32, tag="warm")
    warm_out = pt_pool.tile([P, 1], BF16, tag="warmo")
    nc.gpsimd.memset(warm_sb, 0.0)

    def warmup_pe():
        # ~1.2us of back-to-back PE work so the tensor engine leaves its
        # low/mid p-state before the first projection lands, and a dummy
        # 1-col exp so the Act engine's 1.3us LUT table load happens in
        # the DMA ramp instead of inside the first critical-path exp
        nc.scalar.activation(warm_out, warm_sb, Exp)
        for _ in range(3):
            wps = ps_a.tile([P, 512], F32, tag="kvps")
            for r in range(7):
                nc.tensor.transpose(wps[0:H + 1, r * (H + 1):(r + 1) * (H + 1)],
                                    ident, ident)

    def body(with_dma=True):
        NG = ST // 4     # key-tile groups of 4 (512 cols) per kv projection
        if with_dma:
            load_x()
        warmup_pe()
        # ramp: Q quarters interleaved with the first two kv groups, all
        # gated only on the first four 256-col x DMAs
        # ramp: Q quarters + the first kv group, gated only on the 256-col
        # x DMAs; group g's projection then rides in step g+1 and its
        # V-transpose in step g+2, one step ahead of use
        vts = {}
        q_proj(0, 256)
        q_proj(256, 512)
        vts[0] = kv_proj(0)
        q_proj(512, 768)
        q_proj(768, 1024)
        o_ps0 = ps_o.tile([H + 1, QW], F32, tag="ops")
        prev0 = None
        for g in range(2, NG + 2):
            kv = kv_proj_mm(g - 1, 0, 2) if g - 1 < NG else None
            for j in range(4):
                t = (g - 2) * 4 + j
                pt = s_exp(0, t)
                if kv is not None and j < 3:
                    kv_proj_mm(g - 1, 2 * (j + 1), 2 * (j + 2), kv)
                if j == 0 and g - 2 in vts:
                    v_trans(g - 2, vts.pop(g - 2))
                if prev0 is not None:
                    av(o_ps0, prev0[0], prev0[1])
                prev0 = (t, pt)
            if kv is not None:
                vts[g - 1] = kv_copies(g - 1, kv)
            if g == 2:
                q_proj(1024, 1536)
            if g == 3:
                q_proj(1536, 2048)
        av(o_ps0, prev0[0], prev0[1])

        # qh=1 main loop; qh=0's epilogue blocks ride in its PE/DVE slack
        o_ps1 = ps_o.tile([H + 1, QW], F32, tag="ops")
        epi0 = None
        prev = None
        for t in range(ST):
            pt = s_exp(1, t)
            if t == 0:
                epi0 = epi_start(0, o_ps0)
            if prev is not None:
                av(o_ps1, prev[0], prev[1])
            if 1 <= t <= QW // P:
                epi_block(0, epi0[0], epi0[1], t - 1, on_act=False)
            prev = (t, pt)
        av(o_ps1, prev[0], prev[1])
        epilogue(1, o_ps1)

    import os
    dma_once = bool(int(os.environ.get("KBENCH_DMA_ONCE", "0")))
    if reps == 1:
        body()
    else:
        if dma_once:
            load_x()
        with tc.For_i(0, reps):
            body(with_dma=not dma_once)


def _get_nc():
    if "nc" not in _CACHE:
        _CACHE["nc"] = _build_nc()
    return _CACHE["nc"]


def kernel(x, Wk, Wq, Wv, _trace=False):
    x = np.asarray(x, dtype=np.float32)
    bf = ml_dtypes.bfloat16
    wvk = np.concatenate([np.asarray(Wv, np.float32),
                          np.asarray(Wk, np.float32)], axis=1).astype(bf)
    # fold the 1/sqrt(HEAD_SIZE) score scale into Wq (exact in bf16)
    wq = (np.asarray(Wq, dtype=np.float32) * 0.125).astype(bf)

    in_maps = []
    for c in range(NCORES):
        b, qh = divmod(c, 2)
        xb = x[b]
        if qh:
            xb = np.concatenate([xb[SQ:], xb[:SQ]], axis=0)
        xT = np.ascontiguousarray(xb.T).astype(bf)
        in_maps.append({"xT": xT, "wvk": wvk, "wq": wq})

    nc = _get_nc()
    res = run_bass_kernel_spmd(nc, in_maps, core_ids=list(range(NCORES)),
                               trace=_trace)
    _CACHE["last_result"] = res

    out = np.empty((B, S, H), np.float32)
    for c in range(NCORES):
        b, qh = divmod(c, 2)
        out[b, qh * SQ:(qh + 1) * SQ, :] = res.results[c]["out"]
    return out

